# revision 6
# baseline (speedup 1.0000x reference)
"""Trainium2 Bass kernel for DigitConvolutionalModel forward pass.

Model: x[B,784] -> 3x3 valid conv (single channel) -> flatten[676]
       -> relu(.@W1+b1) -> relu(.@W2+b2) -> .@W3+b3 -> [B,10]

Strategy (v4):
  - Pure data parallel: batch 32768 sharded 8 ways (4096 rows/core);
    weights replicated.
  - conv folds into fc1 (host-side 9-tap sparse weight fold, ~0.02% of
    model FLOPs): fc1 contracts K=784 of pixel-major x against
    W1' = C @ W1. All batch compute runs on device in bf16 (fp32 PSUM).
  - Host supplies x pixel-major bf16 ([784, 4096] per core) and reads the
    output back pixel-major ([10, 4096] per core) — zero-FLOP layout
    changes that remove every on-device transpose.
  - fc1's K=16 leftover chunk (784 = 6*128 + 16) is packed: the three
    h-group tail matmuls run concurrently in disjoint 32-row PE groups
    (tile_position), with x[768:784] and W1'[768:784] replicated at
    partition offsets 0/32/64.
  - fc3 keeps hidden-major [10, 512] output (stationary = W3 chunks of
    only 10 columns -> LDWEIGHTS ~free); bias fused in the ScalarE
    eviction; the tile DMAs straight out to the [10, 4096] buffer.
  - Input + weight DMAs split across both HW-DGE rings (SP + Activation)
    so the prologue is not serialized behind one ~200 GB/s queue.
"""

import sys

for _p in (
    "/opt/trn_rl_repo",
    "/root/.axon_site",
    "/root/.axon_site/_ro/trn_rl_repo",
    "/root/.axon_site/_ro/pypackages",
):
    if _p not in sys.path:
        sys.path.append(_p)

from contextlib import ExitStack

import numpy as np
import ml_dtypes

import concourse.bass as bass
import concourse.tile as tile
from concourse import mybir
from concourse.bass_utils import run_bass_kernel_spmd

F32 = mybir.dt.float32
BF16 = mybir.dt.bfloat16
AFT = mybir.ActivationFunctionType

B_FULL = 32768
N_CORES = 8
B_CORE = B_FULL // N_CORES  # 4096
IMG = 28
OHW = 26
FLAT = OHW * OHW  # 676
NPIX = IMG * IMG  # 784
HID = 300
NCLS = 10

BT = 512  # batch tile (matmul moving free dim)
NBT = B_CORE // BT  # 8

NFULL = 6  # full 128-row pixel chunks; chunk 6 is the 16-row leftover
PIX_CH = [(s, min(128, NPIX - s)) for s in range(0, NPIX, 128)]  # 7 chunks
H_CH = [(s, min(128, HID - s)) for s in range(0, HID, 128)]  # 3 chunks


def _legalize_single_wait(nc):
    """This walrus build accepts only one sync-wait per instruction; move
    extra waits onto NoOps inserted just before, on the same engine."""
    n = 0
    for fn in nc.m.functions:
        for bb in fn.blocks:
            new_insts = []
            for inst in bb.instructions:
                si = inst.sync_info
                if si is not None and si.on_wait and len(si.on_wait) > 1:
                    waits = list(si.on_wait)
                    for w in waits[:-1]:
                        nop = mybir.InstNoOp(
                            name=f"{inst.name}-w{n}",
                            sync_info=mybir.SyncInfo(on_wait=[w], on_update=[]),
                            bass_nofuse=True,
                            engine=inst.engine,
                        )
                        n += 1
                        nc.register_instruction(nop, overwrite=True)
                        new_insts.append(nop)
                    inst.sync_info = mybir.SyncInfo(
                        on_wait=[waits[-1]], on_update=list(si.on_update)
                    )
                new_insts.append(inst)
            bb.instructions = new_insts
    return n


def _emit(ctx: ExitStack, tc: tile.TileContext, xt, w1p_d, w1p6r_d, b1, w2, b2, w3, b3, out):
    nc = tc.nc

    const = ctx.enter_context(tc.tile_pool(name="const", bufs=1))
    ps1 = ctx.enter_context(tc.tile_pool(name="ps1", bufs=3, space="PSUM"))
    ps2p = ctx.enter_context(tc.tile_pool(name="ps2p", bufs=1, space="PSUM"))
    ps3p = ctx.enter_context(tc.tile_pool(name="ps3p", bufs=2, space="PSUM"))
    xp = ctx.enter_context(tc.tile_pool(name="xp", bufs=3))
    hp_ = ctx.enter_context(tc.tile_pool(name="hp", bufs=2))
    obp = ctx.enter_context(tc.tile_pool(name="obp", bufs=4))

    # PE warmup operand: zeros (values are irrelevant for the HAM clock
    # gate; matmuls just need to keep the array busy ~3.4us).
    wz = const.tile([128, 128], BF16, name="wz")
    nc.vector.memset(wz[:, :], 0)

    # --- replicated weights first, split across both HW-DGE rings ---
    w1p = []
    for pc in range(NFULL):
        p0, pw = PIX_CH[pc]
        wt = const.tile([pw, HID], BF16, name=f"w1p{pc}")
        eng = nc.sync if pc % 2 == 0 else nc.scalar
        eng.dma_start(wt[:, :], w1p_d[p0 : p0 + pw, :])
        w1p.append(wt)
    # leftover 16 pixel rows, replicated at partition offsets 0/32/64
    w1p6 = const.tile([80, HID], BF16, name="w1p6")
    nc.sync.dma_start(w1p6[:, :], w1p6r_d[:, :])
    b1s, b2s, w2s, w3s = [], [], [], []
    for hc, (h0, hp) in enumerate(H_CH):
        wt2 = const.tile([hp, HID], BF16, name=f"w2s{hc}")
        eng = nc.scalar if hc % 2 == 0 else nc.sync
        eng.dma_start(wt2[:, :], w2[h0 : h0 + hp, :])
        w2s.append(wt2)
        wt3 = const.tile([hp, NCLS], BF16, name=f"w3s{hc}")
        eng.dma_start(wt3[:, :], w3[h0 : h0 + hp, :])
        w3s.append(wt3)
        bt1 = const.tile([hp, 1], F32, name=f"b1s{hc}")
        nc.sync.dma_start(bt1[:, :], b1[h0 : h0 + hp, :])
        b1s.append(bt1)
        bt2 = const.tile([hp, 1], F32, name=f"b2s{hc}")
        nc.scalar.dma_start(bt2[:, :], b2[h0 : h0 + hp, :])
        b2s.append(bt2)
    b3s = const.tile([NCLS, 1], F32, name="b3s")
    nc.sync.dma_start(b3s[:, :], b3[:, :])

    def load_x(t):
        """DMA the pixel-major bf16 chunks of batch tile t (2 HW rings).
        Chunk 6 (16 rows) is loaded replicated at partitions 0/32/64."""
        c0 = t * BT
        xs = []
        for pc in range(NFULL):
            p0, pw = PIX_CH[pc]
            xn = xp.tile([pw, BT], BF16, name=f"x{pc}", tag=f"x{pc}")
            eng = nc.sync if pc % 2 == 0 else nc.scalar
            eng.dma_start(xn[:, :], xt[p0 : p0 + pw, c0 : c0 + BT])
            xs.append(xn)
        x6 = xp.tile([80, BT], BF16, name="x6", tag="x6")
        for r in range(3):
            eng = (nc.sync, nc.scalar, nc.sync)[r]
            eng.dma_start(
                x6[32 * r : 32 * r + 16, :], xt[768:784, c0 : c0 + BT]
            )
        xs.append(x6)
        return xs

    # warmup burst emitted after the DMA kickoffs so the PE has work while
    # they land
    warm = ps1.tile([128, 512], F32, name="warm", tag="f1")
    for _ in range(30):
        nc.tensor.matmul(
            warm[0:128, 0:128], wz[:, 0:128], wz[:, 0:128],
            start=True, stop=True,
        )

    xts = {0: load_x(0), 1: load_x(1)}

    # --- main batch loop (x DMAs pipelined two tiles ahead) ---
    def compute(xs, c0, off, n, mid=None):
        """fc1->fc2->fc3->store for batch columns [off, off+n) of one tile."""
        # fc1: relu(x @ W1p + b1), output hidden-major [300, n].
        # 6 full-K matmuls per h-group; the three K=16 tail matmuls run
        # concurrently in disjoint 32-row PE groups (tile_position).
        pss = []
        for hc, (h0, hp) in enumerate(H_CH):
            ps = ps1.tile([128, 512], F32, name="psa", tag="f1")
            for pc in range(NFULL):
                p0, pw = PIX_CH[pc]
                nc.tensor.matmul(
                    ps[0:hp, 0:n],
                    w1p[pc][0:pw, h0 : h0 + hp],
                    xs[pc][0:pw, off : off + n],
                    start=(pc == 0),
                    stop=False,
                )
            pss.append(ps)
        for hc, (h0, hp) in enumerate(H_CH):
            r = 32 * hc
            nc.tensor.matmul(
                pss[hc][0:hp, 0:n],
                w1p6[r : r + 16, h0 : h0 + hp],
                xs[NFULL][r : r + 16, off : off + n],
                start=False,
                stop=True,
                tile_position=(r, 0),
            )
        h1 = []
        for hc, (h0, hp) in enumerate(H_CH):
            h = hp_.tile([hp, BT], BF16, name=f"h1_{hc}", tag=f"h1_{hc}")
            nc.scalar.activation(
                h[:, 0:n], pss[hc][0:hp, 0:n], AFT.Relu, bias=b1s[hc][:, :]
            )
            h1.append(h)

        if mid is not None:
            mid()

        # fc2: relu(h1 @ W2 + b2) — k-outer so all m-groups unblock on h1[0]
        ps2 = [
            ps2p.tile([128, 512], F32, name=f"ps2_{g}", tag=f"g{g}")
            for g in range(len(H_CH))
        ]
        for hc, (h0, hp) in enumerate(H_CH):
            for hc2, (g0, gp) in enumerate(H_CH):
                nc.tensor.matmul(
                    ps2[hc2][0:gp, 0:n],
                    w2s[hc][0:hp, g0 : g0 + gp],
                    h1[hc][0:hp, 0:n],
                    start=(hc == 0),
                    stop=(hc == len(H_CH) - 1),
                )
        h2 = []
        for hc2, (g0, gp) in enumerate(H_CH):
            h = hp_.tile([gp, BT], BF16, name=f"h2_{hc2}", tag=f"h2_{hc2}")
            nc.scalar.activation(
                h[:, 0:n], ps2[hc2][0:gp, 0:n], AFT.Relu, bias=b2s[hc2][:, :]
            )
            h2.append(h)

        # fc3: h2 @ W3 + b3 -> [10, n] (10-col stationary, LDW ~free);
        # store hidden-major — the host un-transposes.
        ps = ps3p.tile([NCLS, 512], F32, name="ps3", tag="f3")
        for hc, (h0, hp) in enumerate(H_CH):
            nc.tensor.matmul(
                ps[0:NCLS, 0:n],
                w3s[hc][0:hp, 0:NCLS],
                h2[hc][0:hp, 0:n],
                start=(hc == 0),
                stop=(hc == len(H_CH) - 1),
            )
        ob = obp.tile([NCLS, BT], F32, name="ob", tag="ob")
        nc.scalar.activation(
            ob[:, 0:n], ps[0:NCLS, 0:n], AFT.Identity, bias=b3s[:, :]
        )
        nc.sync.dma_start(out[:, c0 + off : c0 + off + n], ob[:, 0:n])

    for t in range(NBT):
        c0 = t * BT
        xs = xts.pop(t)
        mid = None
        if t + 2 < NBT:
            mid = lambda t=t: xts.__setitem__(t + 2, load_x(t + 2))
        if t == NBT - 1:
            # split the last tile to shorten the serial tail chain
            compute(xs, c0, 0, 256, mid=mid)
            compute(xs, c0, 256, 256)
        else:
            compute(xs, c0, 0, BT, mid=mid)


def _fold_w1(conv_w: np.ndarray, W1: np.ndarray) -> np.ndarray:
    """W1' = C @ W1 via the 9-tap sparse form: 9 scaled slice-adds."""
    W1m = W1.reshape(OHW, OHW, HID)
    out = np.zeros((IMG, IMG, HID), np.float32)
    for dy in range(3):
        for dx in range(3):
            out[dy : dy + OHW, dx : dx + OHW, :] += conv_w[dy, dx] * W1m
    return out.reshape(NPIX, HID)


_NC_CACHE: list = []


def _get_nc():
    if _NC_CACHE:
        return _NC_CACHE[0]
    nc = bass.Bass("TRN2", target_bir_lowering=False, debug=False)
    xt = nc.dram_tensor("xt", [NPIX, B_CORE], BF16, kind="ExternalInput").ap()
    w1p = nc.dram_tensor("w1p", [NPIX, HID], BF16, kind="ExternalInput").ap()
    w1p6r = nc.dram_tensor("w1p6r", [80, HID], BF16, kind="ExternalInput").ap()
    b1 = nc.dram_tensor("b1", [HID, 1], F32, kind="ExternalInput").ap()
    w2 = nc.dram_tensor("w2", [HID, HID], BF16, kind="ExternalInput").ap()
    b2 = nc.dram_tensor("b2", [HID, 1], F32, kind="ExternalInput").ap()
    w3 = nc.dram_tensor("w3", [HID, NCLS], BF16, kind="ExternalInput").ap()
    b3 = nc.dram_tensor("b3", [NCLS, 1], F32, kind="ExternalInput").ap()
    out = nc.dram_tensor("out", [NCLS, B_CORE], F32, kind="ExternalOutput").ap()
    with tile.TileContext(nc) as tc:
        with ExitStack() as ctx:
            _emit(ctx, tc, xt, w1p, w1p6r, b1, w2, b2, w3, b3, out)
    _legalize_single_wait(nc)
    _NC_CACHE.append(nc)
    return nc


def _in_maps(inputs: dict) -> list:
    x = np.asarray(inputs["x"], dtype=np.float32)
    assert x.shape == (B_FULL, NPIX), x.shape
    # pixel-major per-core layout: [8, 784, 4096] bf16 (zero-FLOP reshape)
    xtp = np.ascontiguousarray(
        x.reshape(N_CORES, B_CORE, NPIX).transpose(0, 2, 1)
    ).astype(ml_dtypes.bfloat16)
    bf = ml_dtypes.bfloat16
    w1f = _fold_w1(
        np.asarray(inputs["conv_w"], np.float32),
        np.asarray(inputs["W1"], np.float32),
    ).astype(bf)
    w1p6r = np.zeros((80, HID), bf)
    for r in range(3):
        w1p6r[32 * r : 32 * r + 16] = w1f[768:784]
    common = {
        "w1p": w1f,
        "w1p6r": w1p6r,
        "b1": np.asarray(inputs["b1"], np.float32).reshape(HID, 1),
        "w2": np.ascontiguousarray(np.asarray(inputs["W2"], np.float32)).astype(bf),
        "b2": np.asarray(inputs["b2"], np.float32).reshape(HID, 1),
        "w3": np.ascontiguousarray(np.asarray(inputs["W3"], np.float32)).astype(bf),
        "b3": np.asarray(inputs["b3"], np.float32).reshape(NCLS, 1),
    }
    return [{"xt": xtp[c], **common} for c in range(N_CORES)]


def kernel(**inputs) -> np.ndarray:
    nc = _get_nc()
    res = run_bass_kernel_spmd(nc, _in_maps(inputs), list(range(N_CORES)))
    return np.concatenate(
        [res.results[c]["out"].T for c in range(N_CORES)], axis=0
    )


if __name__ == "__main__":
    rng = np.random.default_rng(0)
    ins = {
        "x": rng.standard_normal((B_FULL, NPIX), dtype=np.float32),
        "conv_w": rng.standard_normal((3, 3), dtype=np.float32) * 0.1,
        "W1": rng.standard_normal((FLAT, HID), dtype=np.float32) * 0.04,
        "b1": np.zeros(HID, np.float32),
        "W2": rng.standard_normal((HID, HID), dtype=np.float32) * 0.06,
        "b2": np.zeros(HID, np.float32),
        "W3": rng.standard_normal((HID, NCLS), dtype=np.float32) * 0.06,
        "b3": np.zeros(NCLS, np.float32),
    }
    y = kernel(**ins)
    # numpy reference with explicit conv
    from numpy.lib.stride_tricks import sliding_window_view

    img = ins["x"].reshape(-1, IMG, IMG)
    win = sliding_window_view(img, (3, 3), axis=(1, 2))
    conv = np.einsum("bijkl,kl->bij", win, ins["conv_w"]).reshape(-1, FLAT)
    h = np.maximum(conv @ ins["W1"] + ins["b1"], 0)
    h = np.maximum(h @ ins["W2"] + ins["b2"], 0)
    ref = h @ ins["W3"] + ins["b3"]
    err = np.abs(y - ref).max() / (np.abs(ref).max() + 1e-9)
    print("max rel err vs numpy:", err)


# revision 8
# speedup vs baseline: 1.0443x; 1.0443x over previous
"""Trainium2 Bass kernel for DigitConvolutionalModel forward pass.

Model: x[B,784] -> 3x3 valid conv (single channel) -> flatten[676]
       -> relu(.@W1+b1) -> relu(.@W2+b2) -> .@W3+b3 -> [B,10]

Strategy (v4):
  - Pure data parallel: batch 32768 sharded 8 ways (4096 rows/core);
    weights replicated.
  - conv folds into fc1 (host-side 9-tap sparse weight fold, ~0.02% of
    model FLOPs): fc1 contracts K=784 of pixel-major x against
    W1' = C @ W1. All batch compute runs on device in bf16 (fp32 PSUM).
  - Host supplies x pixel-major bf16 ([784, 4096] per core) and reads the
    output back pixel-major ([10, 4096] per core) — zero-FLOP layout
    changes that remove every on-device transpose.
  - fc1's K=16 leftover chunk (784 = 6*128 + 16) is packed: the three
    h-group tail matmuls run concurrently in disjoint 32-row PE groups
    (tile_position), with x[768:784] and W1'[768:784] replicated at
    partition offsets 0/32/64.
  - fc3 keeps hidden-major [10, 512] output (stationary = W3 chunks of
    only 10 columns -> LDWEIGHTS ~free); bias fused in the ScalarE
    eviction; the tile DMAs straight out to the [10, 4096] buffer.
  - Input + weight DMAs split across both HW-DGE rings (SP + Activation)
    so the prologue is not serialized behind one ~200 GB/s queue.
"""

import sys

for _p in (
    "/opt/trn_rl_repo",
    "/root/.axon_site",
    "/root/.axon_site/_ro/trn_rl_repo",
    "/root/.axon_site/_ro/pypackages",
):
    if _p not in sys.path:
        sys.path.append(_p)

from contextlib import ExitStack

import numpy as np
import ml_dtypes

import concourse.bass as bass
import concourse.tile as tile
from concourse import mybir
from concourse.bass_utils import run_bass_kernel_spmd

F32 = mybir.dt.float32
BF16 = mybir.dt.bfloat16
AFT = mybir.ActivationFunctionType

B_FULL = 32768
N_CORES = 8
B_CORE = B_FULL // N_CORES  # 4096
IMG = 28
OHW = 26
FLAT = OHW * OHW  # 676
NPIX = IMG * IMG  # 784
HID = 300
NCLS = 10

BT = 512  # batch tile (matmul moving free dim)
NBT = B_CORE // BT  # 8

NFULL = 6  # full 128-row pixel chunks; chunk 6 is the 16-row leftover
PIX_CH = [(s, min(128, NPIX - s)) for s in range(0, NPIX, 128)]  # 7 chunks
H_CH = [(s, min(128, HID - s)) for s in range(0, HID, 128)]  # 3 chunks


def _legalize_single_wait(nc):
    """This walrus build accepts only one sync-wait per instruction; move
    extra waits onto NoOps inserted just before, on the same engine."""
    n = 0
    for fn in nc.m.functions:
        for bb in fn.blocks:
            new_insts = []
            for inst in bb.instructions:
                si = inst.sync_info
                if si is not None and si.on_wait and len(si.on_wait) > 1:
                    waits = list(si.on_wait)
                    for w in waits[:-1]:
                        nop = mybir.InstNoOp(
                            name=f"{inst.name}-w{n}",
                            sync_info=mybir.SyncInfo(on_wait=[w], on_update=[]),
                            bass_nofuse=True,
                            engine=inst.engine,
                        )
                        n += 1
                        nc.register_instruction(nop, overwrite=True)
                        new_insts.append(nop)
                    inst.sync_info = mybir.SyncInfo(
                        on_wait=[waits[-1]], on_update=list(si.on_update)
                    )
                new_insts.append(inst)
            bb.instructions = new_insts
    return n


def _emit(ctx: ExitStack, tc: tile.TileContext, xt, w1p_d, w1p6r_d, b1, w2, b2, w3, b3, out):
    nc = tc.nc

    const = ctx.enter_context(tc.tile_pool(name="const", bufs=1))
    ps1 = ctx.enter_context(tc.tile_pool(name="ps1", bufs=3, space="PSUM"))
    ps2p = ctx.enter_context(tc.tile_pool(name="ps2p", bufs=1, space="PSUM"))
    ps3p = ctx.enter_context(tc.tile_pool(name="ps3p", bufs=2, space="PSUM"))
    xp = ctx.enter_context(tc.tile_pool(name="xp", bufs=3))
    hp_ = ctx.enter_context(tc.tile_pool(name="hp", bufs=2))
    obp = ctx.enter_context(tc.tile_pool(name="obp", bufs=4))

    # PE warmup operand: zeros (values are irrelevant for the HAM clock
    # gate; matmuls just need to keep the array busy ~3.4us).
    wz = const.tile([128, 128], BF16, name="wz")
    nc.vector.memset(wz[:, :], 0)

    # --- replicated weights first, on the Activation HW-DGE ring (idle
    # before the first activations) ---
    w1p = []
    for pc in range(NFULL):
        p0, pw = PIX_CH[pc]
        wt = const.tile([pw, HID], BF16, name=f"w1p{pc}")
        nc.scalar.dma_start(wt[:, :], w1p_d[p0 : p0 + pw, :])
        w1p.append(wt)
    # leftover 16 pixel rows, replicated at partition offsets 0/32/64
    w1p6 = const.tile([80, HID], BF16, name="w1p6")
    nc.scalar.dma_start(w1p6[:, :], w1p6r_d[:, :])
    b1s, b2s, w2s, w3s = [], [], [], []
    for hc, (h0, hp) in enumerate(H_CH):
        wt2 = const.tile([hp, HID], BF16, name=f"w2s{hc}")
        nc.scalar.dma_start(wt2[:, :], w2[h0 : h0 + hp, :])
        w2s.append(wt2)
        wt3 = const.tile([hp, NCLS], BF16, name=f"w3s{hc}")
        nc.scalar.dma_start(wt3[:, :], w3[h0 : h0 + hp, :])
        w3s.append(wt3)
        bt1 = const.tile([hp, 1], F32, name=f"b1s{hc}")
        nc.scalar.dma_start(bt1[:, :], b1[h0 : h0 + hp, :])
        b1s.append(bt1)
        bt2 = const.tile([hp, 1], F32, name=f"b2s{hc}")
        nc.scalar.dma_start(bt2[:, :], b2[h0 : h0 + hp, :])
        b2s.append(bt2)
    b3s = const.tile([NCLS, 1], F32, name="b3s")
    nc.scalar.dma_start(b3s[:, :], b3[:, :])

    def load_x(t):
        """DMA the pixel-major bf16 chunks of batch tile t, split between
        the SP HW-DGE ring and the (otherwise idle) GpSimd SWDGE path.
        Chunk 6 (16 rows) is loaded replicated at partitions 0/32/64."""
        c0 = t * BT
        xs = []
        for pc in range(NFULL):
            p0, pw = PIX_CH[pc]
            xn = xp.tile([pw, BT], BF16, name=f"x{pc}", tag=f"x{pc}")
            eng = nc.sync if pc % 2 == 0 else nc.gpsimd
            eng.dma_start(xn[:, :], xt[p0 : p0 + pw, c0 : c0 + BT])
            xs.append(xn)
        x6 = xp.tile([80, BT], BF16, name="x6", tag="x6")
        for r in range(3):
            eng = (nc.sync, nc.gpsimd, nc.sync)[r]
            eng.dma_start(
                x6[32 * r : 32 * r + 16, :], xt[768:784, c0 : c0 + BT]
            )
        xs.append(x6)
        return xs

    # warmup burst emitted after the DMA kickoffs so the PE has work while
    # they land
    warm = ps1.tile([128, 512], F32, name="warm", tag="f1")
    for _ in range(30):
        nc.tensor.matmul(
            warm[0:128, 0:128], wz[:, 0:128], wz[:, 0:128],
            start=True, stop=True,
        )

    xts = {0: load_x(0), 1: load_x(1)}

    # --- main batch loop (x DMAs pipelined two tiles ahead) ---
    def compute(xs, c0, off, n, mid=None):
        """fc1->fc2->fc3->store for batch columns [off, off+n) of one tile."""
        # fc1: relu(x @ W1p + b1), output hidden-major [300, n].
        # 6 full-K matmuls per h-group; the three K=16 tail matmuls run
        # concurrently in disjoint 32-row PE groups (tile_position).
        pss = []
        for hc, (h0, hp) in enumerate(H_CH):
            ps = ps1.tile([128, 512], F32, name="psa", tag="f1")
            for pc in range(NFULL):
                p0, pw = PIX_CH[pc]
                nc.tensor.matmul(
                    ps[0:hp, 0:n],
                    w1p[pc][0:pw, h0 : h0 + hp],
                    xs[pc][0:pw, off : off + n],
                    start=(pc == 0),
                    stop=False,
                )
            pss.append(ps)
        for hc, (h0, hp) in enumerate(H_CH):
            r = 32 * hc
            nc.tensor.matmul(
                pss[hc][0:hp, 0:n],
                w1p6[r : r + 16, h0 : h0 + hp],
                xs[NFULL][r : r + 16, off : off + n],
                start=False,
                stop=True,
                tile_position=(r, 0),
            )
        h1 = []
        for hc, (h0, hp) in enumerate(H_CH):
            h = hp_.tile([hp, BT], BF16, name=f"h1_{hc}", tag=f"h1_{hc}")
            nc.scalar.activation(
                h[:, 0:n], pss[hc][0:hp, 0:n], AFT.Relu, bias=b1s[hc][:, :]
            )
            h1.append(h)

        if mid is not None:
            mid()

        # fc2: relu(h1 @ W2 + b2) — k-outer so all m-groups unblock on h1[0]
        ps2 = [
            ps2p.tile([128, 512], F32, name=f"ps2_{g}", tag=f"g{g}")
            for g in range(len(H_CH))
        ]
        for hc, (h0, hp) in enumerate(H_CH):
            for hc2, (g0, gp) in enumerate(H_CH):
                nc.tensor.matmul(
                    ps2[hc2][0:gp, 0:n],
                    w2s[hc][0:hp, g0 : g0 + gp],
                    h1[hc][0:hp, 0:n],
                    start=(hc == 0),
                    stop=(hc == len(H_CH) - 1),
                )
        # h2 evictions on DVE (bias-add + relu) to keep ScalarE short
        h2 = []
        for hc2, (g0, gp) in enumerate(H_CH):
            h = hp_.tile([gp, BT], BF16, name=f"h2_{hc2}", tag=f"h2_{hc2}")
            nc.vector.tensor_scalar(
                h[:, 0:n], ps2[hc2][0:gp, 0:n], b2s[hc2][:, :], 0.0,
                mybir.AluOpType.add, mybir.AluOpType.max,
            )
            h2.append(h)

        # fc3: h2 @ W3 + b3 -> [10, n] (10-col stationary, LDW ~free);
        # store hidden-major — the host un-transposes.
        ps = ps3p.tile([NCLS, 512], F32, name="ps3", tag="f3")
        for hc, (h0, hp) in enumerate(H_CH):
            nc.tensor.matmul(
                ps[0:NCLS, 0:n],
                w3s[hc][0:hp, 0:NCLS],
                h2[hc][0:hp, 0:n],
                start=(hc == 0),
                stop=(hc == len(H_CH) - 1),
            )
        ob = obp.tile([NCLS, BT], F32, name="ob", tag="ob")
        nc.scalar.activation(
            ob[:, 0:n], ps[0:NCLS, 0:n], AFT.Identity, bias=b3s[:, :]
        )
        nc.sync.dma_start(out[:, c0 + off : c0 + off + n], ob[:, 0:n])

    for t in range(NBT):
        c0 = t * BT
        xs = xts.pop(t)
        mid = None
        if t + 2 < NBT:
            mid = lambda t=t: xts.__setitem__(t + 2, load_x(t + 2))
        if t == NBT - 1:
            # split the last tile to shorten the serial tail chain
            compute(xs, c0, 0, 256, mid=mid)
            compute(xs, c0, 256, 256)
        else:
            compute(xs, c0, 0, BT, mid=mid)


def _fold_w1(conv_w: np.ndarray, W1: np.ndarray) -> np.ndarray:
    """W1' = C @ W1 via the 9-tap sparse form: 9 scaled slice-adds."""
    W1m = W1.reshape(OHW, OHW, HID)
    out = np.zeros((IMG, IMG, HID), np.float32)
    for dy in range(3):
        for dx in range(3):
            out[dy : dy + OHW, dx : dx + OHW, :] += conv_w[dy, dx] * W1m
    return out.reshape(NPIX, HID)


_NC_CACHE: list = []


def _get_nc():
    if _NC_CACHE:
        return _NC_CACHE[0]
    nc = bass.Bass("TRN2", target_bir_lowering=False, debug=False)
    xt = nc.dram_tensor("xt", [NPIX, B_CORE], BF16, kind="ExternalInput").ap()
    w1p = nc.dram_tensor("w1p", [NPIX, HID], BF16, kind="ExternalInput").ap()
    w1p6r = nc.dram_tensor("w1p6r", [80, HID], BF16, kind="ExternalInput").ap()
    b1 = nc.dram_tensor("b1", [HID, 1], F32, kind="ExternalInput").ap()
    w2 = nc.dram_tensor("w2", [HID, HID], BF16, kind="ExternalInput").ap()
    b2 = nc.dram_tensor("b2", [HID, 1], F32, kind="ExternalInput").ap()
    w3 = nc.dram_tensor("w3", [HID, NCLS], BF16, kind="ExternalInput").ap()
    b3 = nc.dram_tensor("b3", [NCLS, 1], F32, kind="ExternalInput").ap()
    out = nc.dram_tensor("out", [NCLS, B_CORE], F32, kind="ExternalOutput").ap()
    with tile.TileContext(nc) as tc:
        with ExitStack() as ctx:
            _emit(ctx, tc, xt, w1p, w1p6r, b1, w2, b2, w3, b3, out)
    _legalize_single_wait(nc)
    _NC_CACHE.append(nc)
    return nc


def _in_maps(inputs: dict) -> list:
    x = np.asarray(inputs["x"], dtype=np.float32)
    assert x.shape == (B_FULL, NPIX), x.shape
    # pixel-major per-core layout: [8, 784, 4096] bf16 (zero-FLOP reshape)
    xtp = np.ascontiguousarray(
        x.reshape(N_CORES, B_CORE, NPIX).transpose(0, 2, 1)
    ).astype(ml_dtypes.bfloat16)
    bf = ml_dtypes.bfloat16
    w1f = _fold_w1(
        np.asarray(inputs["conv_w"], np.float32),
        np.asarray(inputs["W1"], np.float32),
    ).astype(bf)
    w1p6r = np.zeros((80, HID), bf)
    for r in range(3):
        w1p6r[32 * r : 32 * r + 16] = w1f[768:784]
    common = {
        "w1p": w1f,
        "w1p6r": w1p6r,
        "b1": np.asarray(inputs["b1"], np.float32).reshape(HID, 1),
        "w2": np.ascontiguousarray(np.asarray(inputs["W2"], np.float32)).astype(bf),
        "b2": np.asarray(inputs["b2"], np.float32).reshape(HID, 1),
        "w3": np.ascontiguousarray(np.asarray(inputs["W3"], np.float32)).astype(bf),
        "b3": np.asarray(inputs["b3"], np.float32).reshape(NCLS, 1),
    }
    return [{"xt": xtp[c], **common} for c in range(N_CORES)]


def kernel(**inputs) -> np.ndarray:
    nc = _get_nc()
    res = run_bass_kernel_spmd(nc, _in_maps(inputs), list(range(N_CORES)))
    return np.concatenate(
        [res.results[c]["out"].T for c in range(N_CORES)], axis=0
    )


if __name__ == "__main__":
    rng = np.random.default_rng(0)
    ins = {
        "x": rng.standard_normal((B_FULL, NPIX), dtype=np.float32),
        "conv_w": rng.standard_normal((3, 3), dtype=np.float32) * 0.1,
        "W1": rng.standard_normal((FLAT, HID), dtype=np.float32) * 0.04,
        "b1": np.zeros(HID, np.float32),
        "W2": rng.standard_normal((HID, HID), dtype=np.float32) * 0.06,
        "b2": np.zeros(HID, np.float32),
        "W3": rng.standard_normal((HID, NCLS), dtype=np.float32) * 0.06,
        "b3": np.zeros(NCLS, np.float32),
    }
    y = kernel(**ins)
    # numpy reference with explicit conv
    from numpy.lib.stride_tricks import sliding_window_view

    img = ins["x"].reshape(-1, IMG, IMG)
    win = sliding_window_view(img, (3, 3), axis=(1, 2))
    conv = np.einsum("bijkl,kl->bij", win, ins["conv_w"]).reshape(-1, FLAT)
    h = np.maximum(conv @ ins["W1"] + ins["b1"], 0)
    h = np.maximum(h @ ins["W2"] + ins["b2"], 0)
    ref = h @ ins["W3"] + ins["b3"]
    err = np.abs(y - ref).max() / (np.abs(ref).max() + 1e-9)
    print("max rel err vs numpy:", err)


# revision 14
# speedup vs baseline: 1.1258x; 1.0780x over previous
"""Trainium2 Bass kernel for DigitConvolutionalModel forward pass.

Model: x[B,784] -> 3x3 valid conv (single channel) -> flatten[676]
       -> relu(.@W1+b1) -> relu(.@W2+b2) -> .@W3+b3 -> [B,10]

Strategy (v4):
  - Pure data parallel: batch 32768 sharded 8 ways (4096 rows/core);
    weights replicated.
  - conv folds into fc1 (host-side 9-tap sparse weight fold, ~0.02% of
    model FLOPs): fc1 contracts K=784 of pixel-major x against
    W1' = C @ W1. All batch compute runs on device in bf16 (fp32 PSUM).
  - Host supplies x pixel-major bf16 ([784, 4096] per core) and reads the
    output back pixel-major ([10, 4096] per core) — zero-FLOP layout
    changes that remove every on-device transpose.
  - fc1's K=16 leftover chunk (784 = 6*128 + 16) is packed: the three
    h-group tail matmuls run concurrently in disjoint 32-row PE groups
    (tile_position), with x[768:784] and W1'[768:784] replicated at
    partition offsets 0/32/64.
  - fc3 keeps hidden-major [10, 512] output (stationary = W3 chunks of
    only 10 columns -> LDWEIGHTS ~free); bias fused in the ScalarE
    eviction; the tile DMAs straight out to the [10, 4096] buffer.
  - Input + weight DMAs split across both HW-DGE rings (SP + Activation)
    so the prologue is not serialized behind one ~200 GB/s queue.
"""

import sys

for _p in (
    "/opt/trn_rl_repo",
    "/root/.axon_site",
    "/root/.axon_site/_ro/trn_rl_repo",
    "/root/.axon_site/_ro/pypackages",
):
    if _p not in sys.path:
        sys.path.append(_p)

from contextlib import ExitStack

import numpy as np
import ml_dtypes

import concourse.bass as bass
import concourse.tile as tile
from concourse import mybir
from concourse.bass_utils import run_bass_kernel_spmd

F32 = mybir.dt.float32
BF16 = mybir.dt.bfloat16
AFT = mybir.ActivationFunctionType

B_FULL = 32768
N_CORES = 8
B_CORE = B_FULL // N_CORES  # 4096
IMG = 28
OHW = 26
FLAT = OHW * OHW  # 676
NPIX = IMG * IMG  # 784
HID = 300
NCLS = 10

BT = 512  # batch tile (matmul moving free dim)
NBT = B_CORE // BT  # 8

NFULL = 6  # full 128-row pixel chunks; chunk 6 is the 16-row leftover
PIX_CH = [(s, min(128, NPIX - s)) for s in range(0, NPIX, 128)]  # 7 chunks
H_CH = [(s, min(128, HID - s)) for s in range(0, HID, 128)]  # 3 chunks


def _legalize_single_wait(nc):
    """This walrus build accepts only one sync-wait per instruction; move
    extra waits onto NoOps inserted just before, on the same engine."""
    n = 0
    for fn in nc.m.functions:
        for bb in fn.blocks:
            new_insts = []
            for inst in bb.instructions:
                si = inst.sync_info
                if si is not None and si.on_wait and len(si.on_wait) > 1:
                    waits = list(si.on_wait)
                    for w in waits[:-1]:
                        nop = mybir.InstNoOp(
                            name=f"{inst.name}-w{n}",
                            sync_info=mybir.SyncInfo(on_wait=[w], on_update=[]),
                            bass_nofuse=True,
                            engine=inst.engine,
                        )
                        n += 1
                        nc.register_instruction(nop, overwrite=True)
                        new_insts.append(nop)
                    inst.sync_info = mybir.SyncInfo(
                        on_wait=[waits[-1]], on_update=list(si.on_update)
                    )
                new_insts.append(inst)
            bb.instructions = new_insts
    return n


def _emit(ctx: ExitStack, tc: tile.TileContext, xt, x6r, wpk_d, bpk_d, out):
    nc = tc.nc

    const = ctx.enter_context(tc.tile_pool(name="const", bufs=1))
    ps1 = ctx.enter_context(tc.tile_pool(name="ps1", bufs=3, space="PSUM"))
    ps2p = ctx.enter_context(tc.tile_pool(name="ps2p", bufs=1, space="PSUM"))
    ps3p = ctx.enter_context(tc.tile_pool(name="ps3p", bufs=2, space="PSUM"))
    xp = ctx.enter_context(tc.tile_pool(name="xp", bufs=3))
    hp_ = ctx.enter_context(tc.tile_pool(name="hp", bufs=2))
    obp = ctx.enter_context(tc.tile_pool(name="obp", bufs=4))

    # PE warmup operand: zeros (values are irrelevant for the HAM clock
    # gate; matmuls just need to keep the array busy ~3.4us).
    wz = const.tile([128, 128], BF16, name="wz")
    nc.vector.memset(wz[:, :], 0)

    # --- replicated weights: 3 packed DMAs on the Activation HW-DGE ring
    # (idle before the first activations). Per-DMA ring cost is ~600ns
    # regardless of size, so everything ships in wide packed tiles. ---
    # wpk layout (host-packed, bf16): cols [0,1800) = w1p chunks 0-5,
    # [1800,2100) = w1p6 (replicated at partition offsets 0/32/64),
    # [2100,3000) = w2 chunks, [3000,3030) = w3 chunks.
    wpk = const.tile([128, 3030], BF16, name="wpk")
    nc.scalar.dma_start(wpk[:, :], wpk_d[:, :])
    w1p = [wpk[0:pw, pc * HID : pc * HID + HID] for pc, (p0, pw) in enumerate(PIX_CH[:NFULL])]
    w1p6 = wpk[0:80, NFULL * HID : NFULL * HID + HID]
    w2s = [wpk[0:hp, 2100 + hc * HID : 2100 + (hc + 1) * HID] for hc, (h0, hp) in enumerate(H_CH)]
    w3s = [wpk[0:hp, 3000 + hc * NCLS : 3000 + (hc + 1) * NCLS] for hc, (h0, hp) in enumerate(H_CH)]
    # bias pack (f32): cols 0-2 = b1 chunks, 3-5 = b2 chunks, 6 = b3
    bpk = const.tile([128, 7], F32, name="bpk")
    nc.scalar.dma_start(bpk[:, :], bpk_d[:, :])
    b1s = [bpk[0:hp, hc : hc + 1] for hc, (h0, hp) in enumerate(H_CH)]
    b2s = [bpk[0:hp, 3 + hc : 4 + hc] for hc, (h0, hp) in enumerate(H_CH)]
    b3s = bpk[0:NCLS, 6:7]

    def load_x(t):
        """DMA batch tile t in 3 transfers: chunks 0-2 (SP ring), 3-5
        (GpSimd SWDGE), and the pre-replicated 16-row leftover (SP)."""
        c0 = t * BT
        xa = xp.tile([128, 3 * BT], BF16, name="xa", tag="xa")
        nc.sync.dma_start(
            xa[:, :].rearrange("p (pc b) -> p pc b", pc=3),
            xt[0:384, c0 : c0 + BT].rearrange("(pc p) b -> p pc b", pc=3),
        )
        xb = xp.tile([128, 3 * BT], BF16, name="xb", tag="xb")
        nc.gpsimd.dma_start(
            xb[:, :].rearrange("p (pc b) -> p pc b", pc=3),
            xt[384:768, c0 : c0 + BT].rearrange("(pc p) b -> p pc b", pc=3),
        )
        x6 = xp.tile([80, BT], BF16, name="x6", tag="x6")
        nc.gpsimd.dma_start(x6[:, :], x6r[:, c0 : c0 + BT])
        xs = [xa[:, pc * BT : (pc + 1) * BT] for pc in range(3)]
        xs += [xb[:, pc * BT : (pc + 1) * BT] for pc in range(3)]
        xs.append(x6)
        return xs

    # warmup burst emitted after the DMA kickoffs so the PE has work while
    # they land
    warm = ps1.tile([128, 512], F32, name="warm", tag="f1")
    for _ in range(30):
        nc.tensor.matmul(
            warm[0:128, 0:128], wz[:, 0:128], wz[:, 0:128],
            start=True, stop=True,
        )

    xts = {0: load_x(0), 1: load_x(1)}

    # --- main batch loop (x DMAs pipelined two tiles ahead) ---
    def compute(xs, c0, off, n, mid=None):
        """fc1->fc2->fc3->store for batch columns [off, off+n) of one tile."""
        # fc1: relu(x @ W1p + b1), output hidden-major [300, n].
        # 6 full-K matmuls per h-group; the three K=16 tail matmuls run
        # concurrently in disjoint 32-row PE groups (tile_position).
        pss = []
        for hc, (h0, hp) in enumerate(H_CH):
            ps = ps1.tile([128, 512], F32, name="psa", tag="f1")
            for pc in range(NFULL):
                p0, pw = PIX_CH[pc]
                nc.tensor.matmul(
                    ps[0:hp, 0:n],
                    w1p[pc][0:pw, h0 : h0 + hp],
                    xs[pc][0:pw, off : off + n],
                    start=(pc == 0),
                    stop=False,
                )
            pss.append(ps)
        for hc, (h0, hp) in enumerate(H_CH):
            r = 32 * hc
            nc.tensor.matmul(
                pss[hc][0:hp, 0:n],
                w1p6[r : r + 16, h0 : h0 + hp],
                xs[NFULL][r : r + 16, off : off + n],
                start=False,
                stop=True,
                tile_position=(r, 0),
            )
        h1 = []
        for hc, (h0, hp) in enumerate(H_CH):
            h = hp_.tile([hp, BT], BF16, name=f"h1_{hc}", tag=f"h1_{hc}")
            nc.scalar.activation(
                h[:, 0:n], pss[hc][0:hp, 0:n], AFT.Relu, bias=b1s[hc][:, :]
            )
            h1.append(h)

        if mid is not None:
            mid()

        # fc2: relu(h1 @ W2 + b2) — m-outer: consecutive matmuls share a
        # PSUM bank, which keeps LDWEIGHTS hidden (bank switches expose it)
        ps2 = [
            ps2p.tile([128, 512], F32, name=f"ps2_{g}", tag=f"g{g}")
            for g in range(len(H_CH))
        ]
        for hc2, (g0, gp) in enumerate(H_CH):
            for hc, (h0, hp) in enumerate(H_CH):
                nc.tensor.matmul(
                    ps2[hc2][0:gp, 0:n],
                    w2s[hc][0:hp, g0 : g0 + gp],
                    h1[hc][0:hp, 0:n],
                    start=(hc == 0),
                    stop=(hc == len(H_CH) - 1),
                )
        # h2 evictions on DVE (bias-add + relu) to keep ScalarE short
        h2 = []
        for hc2, (g0, gp) in enumerate(H_CH):
            h = hp_.tile([gp, BT], BF16, name=f"h2_{hc2}", tag=f"h2_{hc2}")
            nc.vector.tensor_scalar(
                h[:, 0:n], ps2[hc2][0:gp, 0:n], b2s[hc2][:, :], 0.0,
                mybir.AluOpType.add, mybir.AluOpType.max,
            )
            h2.append(h)

        # fc3: h2 @ W3 + b3 -> [10, n] (10-col stationary, LDW ~free);
        # store hidden-major — the host un-transposes.
        ps = ps3p.tile([NCLS, 512], F32, name="ps3", tag="f3")
        for hc, (h0, hp) in enumerate(H_CH):
            nc.tensor.matmul(
                ps[0:NCLS, 0:n],
                w3s[hc][0:hp, 0:NCLS],
                h2[hc][0:hp, 0:n],
                start=(hc == 0),
                stop=(hc == len(H_CH) - 1),
            )
        ob = obp.tile([NCLS, BT], F32, name="ob", tag="ob")
        nc.scalar.activation(
            ob[:, 0:n], ps[0:NCLS, 0:n], AFT.Identity, bias=b3s[:, :]
        )
        nc.sync.dma_start(out[:, c0 + off : c0 + off + n], ob[:, 0:n])

    for t in range(NBT):
        c0 = t * BT
        xs = xts.pop(t)
        mid = None
        if t + 2 < NBT:
            mid = lambda t=t: xts.__setitem__(t + 2, load_x(t + 2))
        if t == NBT - 1:
            # split the last tile to shorten the serial tail chain
            compute(xs, c0, 0, 256, mid=mid)
            compute(xs, c0, 256, 256)
        else:
            compute(xs, c0, 0, BT, mid=mid)


def _fold_w1(conv_w: np.ndarray, W1: np.ndarray) -> np.ndarray:
    """W1' = C @ W1 via the 9-tap sparse form: 9 scaled slice-adds."""
    W1m = W1.reshape(OHW, OHW, HID)
    out = np.zeros((IMG, IMG, HID), np.float32)
    for dy in range(3):
        for dx in range(3):
            out[dy : dy + OHW, dx : dx + OHW, :] += conv_w[dy, dx] * W1m
    return out.reshape(NPIX, HID)


_NC_CACHE: list = []


def _get_nc():
    if _NC_CACHE:
        return _NC_CACHE[0]
    nc = bass.Bass("TRN2", target_bir_lowering=False, debug=False)
    xt = nc.dram_tensor("xt", [NPIX, B_CORE], BF16, kind="ExternalInput").ap()
    x6r = nc.dram_tensor("x6r", [80, B_CORE], BF16, kind="ExternalInput").ap()
    wpk = nc.dram_tensor("wpk", [128, 3030], BF16, kind="ExternalInput").ap()
    bpk = nc.dram_tensor("bpk", [128, 7], F32, kind="ExternalInput").ap()
    out = nc.dram_tensor("out", [NCLS, B_CORE], F32, kind="ExternalOutput").ap()
    with tile.TileContext(nc) as tc:
        with ExitStack() as ctx:
            _emit(ctx, tc, xt, x6r, wpk, bpk, out)
    _legalize_single_wait(nc)
    _NC_CACHE.append(nc)
    return nc


def _in_maps(inputs: dict) -> list:
    x = np.asarray(inputs["x"], dtype=np.float32)
    assert x.shape == (B_FULL, NPIX), x.shape
    # pixel-major per-core layout: [8, 784, 4096] bf16 (zero-FLOP reshape)
    xtp = np.ascontiguousarray(
        x.reshape(N_CORES, B_CORE, NPIX).transpose(0, 2, 1)
    ).astype(ml_dtypes.bfloat16)
    bf = ml_dtypes.bfloat16
    w1f = _fold_w1(
        np.asarray(inputs["conv_w"], np.float32),
        np.asarray(inputs["W1"], np.float32),
    ).astype(bf)
    W2 = np.asarray(inputs["W2"], np.float32)
    W3 = np.asarray(inputs["W3"], np.float32)
    # packed weight tile: w1p chunks 0-5 | w1p6 (replicated) | w2 | w3
    wpk = np.zeros((128, 3030), bf)
    for pc in range(NFULL):
        wpk[:, pc * HID : (pc + 1) * HID] = w1f[pc * 128 : (pc + 1) * 128]
    for r in range(3):
        wpk[32 * r : 32 * r + 16, NFULL * HID : NFULL * HID + HID] = w1f[768:784]
    for hc, (h0, hp) in enumerate(H_CH):
        wpk[0:hp, 2100 + hc * HID : 2100 + (hc + 1) * HID] = W2[h0 : h0 + hp].astype(bf)
        wpk[0:hp, 3000 + hc * NCLS : 3000 + (hc + 1) * NCLS] = W3[h0 : h0 + hp].astype(bf)
    bpk = np.zeros((128, 7), np.float32)
    b1 = np.asarray(inputs["b1"], np.float32)
    b2 = np.asarray(inputs["b2"], np.float32)
    for hc, (h0, hp) in enumerate(H_CH):
        bpk[0:hp, hc] = b1[h0 : h0 + hp]
        bpk[0:hp, 3 + hc] = b2[h0 : h0 + hp]
    bpk[0:NCLS, 6] = np.asarray(inputs["b3"], np.float32)
    # 16-row x leftover, replicated at partition offsets 0/32/64
    x6r = np.zeros((N_CORES, 80, B_CORE), bf)
    for r in range(3):
        x6r[:, 32 * r : 32 * r + 16] = xtp[:, 768:784]
    common = {"wpk": wpk, "bpk": bpk}
    return [{"xt": xtp[c], "x6r": x6r[c], **common} for c in range(N_CORES)]


def kernel(**inputs) -> np.ndarray:
    nc = _get_nc()
    res = run_bass_kernel_spmd(nc, _in_maps(inputs), list(range(N_CORES)))
    return np.concatenate(
        [res.results[c]["out"].T for c in range(N_CORES)], axis=0
    )


if __name__ == "__main__":
    rng = np.random.default_rng(0)
    ins = {
        "x": rng.standard_normal((B_FULL, NPIX), dtype=np.float32),
        "conv_w": rng.standard_normal((3, 3), dtype=np.float32) * 0.1,
        "W1": rng.standard_normal((FLAT, HID), dtype=np.float32) * 0.04,
        "b1": np.zeros(HID, np.float32),
        "W2": rng.standard_normal((HID, HID), dtype=np.float32) * 0.06,
        "b2": np.zeros(HID, np.float32),
        "W3": rng.standard_normal((HID, NCLS), dtype=np.float32) * 0.06,
        "b3": np.zeros(NCLS, np.float32),
    }
    y = kernel(**ins)
    # numpy reference with explicit conv
    from numpy.lib.stride_tricks import sliding_window_view

    img = ins["x"].reshape(-1, IMG, IMG)
    win = sliding_window_view(img, (3, 3), axis=(1, 2))
    conv = np.einsum("bijkl,kl->bij", win, ins["conv_w"]).reshape(-1, FLAT)
    h = np.maximum(conv @ ins["W1"] + ins["b1"], 0)
    h = np.maximum(h @ ins["W2"] + ins["b2"], 0)
    ref = h @ ins["W3"] + ins["b3"]
    err = np.abs(y - ref).max() / (np.abs(ref).max() + 1e-9)
    print("max rel err vs numpy:", err)


# revision 19
# speedup vs baseline: 1.1963x; 1.0626x over previous
"""Trainium2 Bass kernel for DigitConvolutionalModel forward pass.

Model: x[B,784] -> 3x3 valid conv (single channel) -> flatten[676]
       -> relu(.@W1+b1) -> relu(.@W2+b2) -> .@W3+b3 -> [B,10]

Strategy (v4):
  - Pure data parallel: batch 32768 sharded 8 ways (4096 rows/core);
    weights replicated.
  - conv folds into fc1 (host-side 9-tap sparse weight fold, ~0.02% of
    model FLOPs): fc1 contracts K=784 of pixel-major x against
    W1' = C @ W1. All batch compute runs on device in bf16 (fp32 PSUM).
  - Host supplies x pixel-major bf16 ([784, 4096] per core) and reads the
    output back pixel-major ([10, 4096] per core) — zero-FLOP layout
    changes that remove every on-device transpose.
  - fc1's K=16 leftover chunk (784 = 6*128 + 16) is packed: the three
    h-group tail matmuls run concurrently in disjoint 32-row PE groups
    (tile_position), with x[768:784] and W1'[768:784] replicated at
    partition offsets 0/32/64.
  - fc3 keeps hidden-major [10, 512] output (stationary = W3 chunks of
    only 10 columns -> LDWEIGHTS ~free); bias fused in the ScalarE
    eviction; the tile DMAs straight out to the [10, 4096] buffer.
  - Input + weight DMAs split across both HW-DGE rings (SP + Activation)
    so the prologue is not serialized behind one ~200 GB/s queue.
"""

import sys

for _p in (
    "/opt/trn_rl_repo",
    "/root/.axon_site",
    "/root/.axon_site/_ro/trn_rl_repo",
    "/root/.axon_site/_ro/pypackages",
):
    if _p not in sys.path:
        sys.path.append(_p)

from contextlib import ExitStack

import numpy as np
import ml_dtypes

import concourse.bass as bass
import concourse.tile as tile
from concourse import mybir
from concourse.bass_utils import run_bass_kernel_spmd

F32 = mybir.dt.float32
BF16 = mybir.dt.bfloat16
AFT = mybir.ActivationFunctionType

B_FULL = 32768
N_CORES = 8
B_CORE = B_FULL // N_CORES  # 4096
IMG = 28
OHW = 26
FLAT = OHW * OHW  # 676
NPIX = IMG * IMG  # 784
HID = 300
NCLS = 10

BT = 512  # batch tile (matmul moving free dim)
NBT = B_CORE // BT  # 8

NFULL = 6  # full 128-row pixel chunks; chunk 6 is the 16-row leftover
PIX_CH = [(s, min(128, NPIX - s)) for s in range(0, NPIX, 128)]  # 7 chunks
H_CH = [(s, min(128, HID - s)) for s in range(0, HID, 128)]  # 3 chunks


def _legalize_single_wait(nc):
    """This walrus build accepts only one sync-wait per instruction; move
    extra waits onto NoOps inserted just before, on the same engine."""
    n = 0
    for fn in nc.m.functions:
        for bb in fn.blocks:
            new_insts = []
            for inst in bb.instructions:
                si = inst.sync_info
                if si is not None and si.on_wait and len(si.on_wait) > 1:
                    waits = list(si.on_wait)
                    for w in waits[:-1]:
                        nop = mybir.InstNoOp(
                            name=f"{inst.name}-w{n}",
                            sync_info=mybir.SyncInfo(on_wait=[w], on_update=[]),
                            bass_nofuse=True,
                            engine=inst.engine,
                        )
                        n += 1
                        nc.register_instruction(nop, overwrite=True)
                        new_insts.append(nop)
                    inst.sync_info = mybir.SyncInfo(
                        on_wait=[waits[-1]], on_update=list(si.on_update)
                    )
                new_insts.append(inst)
            bb.instructions = new_insts
    return n


def _emit(ctx: ExitStack, tc: tile.TileContext, xt, wpk_d, bpk_d, out):
    nc = tc.nc

    const = ctx.enter_context(tc.tile_pool(name="const", bufs=1))
    ps1 = ctx.enter_context(tc.tile_pool(name="ps1", bufs=3, space="PSUM"))
    ps2p = ctx.enter_context(tc.tile_pool(name="ps2p", bufs=1, space="PSUM"))
    ps3p = ctx.enter_context(tc.tile_pool(name="ps3p", bufs=2, space="PSUM"))
    xp = ctx.enter_context(tc.tile_pool(name="xp", bufs=3))
    hp_ = ctx.enter_context(tc.tile_pool(name="hp", bufs=2))
    obp = ctx.enter_context(tc.tile_pool(name="obp", bufs=4))

    # PE warmup operand: zeros (values are irrelevant for the HAM clock
    # gate; matmuls just need to keep the array busy ~3.4us).
    wz = const.tile([128, 128], BF16, name="wz")
    nc.vector.memset(wz[:, :], 0)

    # --- replicated weights: 3 packed DMAs on the Activation HW-DGE ring
    # (idle before the first activations). Per-DMA ring cost is ~600ns
    # regardless of size, so everything ships in wide packed tiles. ---
    # wpk layout (host-packed, bf16): cols [0,1800) = w1p chunks 0-5,
    # [1800,2100) = w1p6 (replicated at partition offsets 0/32/64),
    # [2100,3000) = w2 chunks, [3000,3030) = w3 chunks.
    wpk = const.tile([128, 3030], BF16, name="wpk")
    nc.scalar.dma_start(wpk[:, :], wpk_d[:, :])
    w1p = [wpk[0:pw, pc * HID : pc * HID + HID] for pc, (p0, pw) in enumerate(PIX_CH[:NFULL])]
    w1p6 = wpk[0:80, NFULL * HID : NFULL * HID + HID]
    w2s = [wpk[0:hp, 2100 + hc * HID : 2100 + (hc + 1) * HID] for hc, (h0, hp) in enumerate(H_CH)]
    w3s = [wpk[0:hp, 3000 + hc * NCLS : 3000 + (hc + 1) * NCLS] for hc, (h0, hp) in enumerate(H_CH)]
    # bias pack (f32): cols 0-2 = b1 chunks, 3-5 = b2 chunks, 6 = b3
    bpk = const.tile([128, 7], F32, name="bpk")
    nc.scalar.dma_start(bpk[:, :], bpk_d[:, :])
    b1s = [bpk[0:hp, hc : hc + 1] for hc, (h0, hp) in enumerate(H_CH)]
    b2s = [bpk[0:hp, 3 + hc : 4 + hc] for hc, (h0, hp) in enumerate(H_CH)]
    b3s = bpk[0:NCLS, 6:7]

    def load_x(t):
        """DMA batch tile t in 3 transfers: chunks 0-2 (SP ring), 3-5
        (GpSimd SWDGE), and the pre-replicated 16-row leftover (SP)."""
        c0 = t * BT
        xa = xp.tile([128, 3 * BT], BF16, name="xa", tag="xa")
        nc.sync.dma_start(
            xa[:, :].rearrange("p (pc b) -> p pc b", pc=3),
            xt[0:384, c0 : c0 + BT].rearrange("(pc p) b -> p pc b", pc=3),
        )
        xb = xp.tile([128, 3 * BT], BF16, name="xb", tag="xb")
        nc.gpsimd.dma_start(
            xb[:, :].rearrange("p (pc b) -> p pc b", pc=3),
            xt[384:768, c0 : c0 + BT].rearrange("(pc p) b -> p pc b", pc=3),
        )
        x6 = xp.tile([16, BT], BF16, name="x6", tag="x6")
        nc.sync.dma_start(x6[:, :], xt[768:784, c0 : c0 + BT])
        xs = [xa[:, pc * BT : (pc + 1) * BT] for pc in range(3)]
        xs += [xb[:, pc * BT : (pc + 1) * BT] for pc in range(3)]
        xs.append(x6)
        return xs

    # warmup burst emitted after the DMA kickoffs so the PE has work while
    # they land
    warm = ps1.tile([128, 512], F32, name="warm", tag="f1")
    for _ in range(30):
        nc.tensor.matmul(
            warm[0:128, 0:128], wz[:, 0:128], wz[:, 0:128],
            start=True, stop=True,
        )

    xts = {0: load_x(0), 1: load_x(1)}

    # --- main batch loop (x DMAs pipelined two tiles ahead) ---
    def compute(xs, c0, off, n, mid=None):
        """fc1->fc2->fc3->store for batch columns [off, off+n) of one tile."""
        # fc1: relu(x @ W1p + b1), output hidden-major [300, n]; each
        # h-group's 7 matmuls stay bank-contiguous and its ACT eviction
        # starts while the next group runs on the PE.
        h1 = []
        for hc, (h0, hp) in enumerate(H_CH):
            ps = ps1.tile([128, 512], F32, name="psa", tag="f1")
            for pc, (p0, pw) in enumerate(PIX_CH):
                nc.tensor.matmul(
                    ps[0:hp, 0:n],
                    w1p[pc][0:pw, h0 : h0 + hp] if pc < NFULL
                    else w1p6[0:16, h0 : h0 + hp],
                    xs[pc][0:pw, off : off + n],
                    start=(pc == 0),
                    stop=(pc == len(PIX_CH) - 1),
                )
            h = hp_.tile([hp, BT], BF16, name=f"h1_{hc}", tag=f"h1_{hc}")
            nc.scalar.activation(
                h[:, 0:n], ps[0:hp, 0:n], AFT.Relu, bias=b1s[hc][:, :]
            )
            h1.append(h)

        if mid is not None:
            mid()

        # fc2: relu(h1 @ W2 + b2) — m-outer: consecutive matmuls share a
        # PSUM bank, which keeps LDWEIGHTS hidden (bank switches expose it)
        ps2 = [
            ps2p.tile([128, 512], F32, name=f"ps2_{g}", tag=f"g{g}")
            for g in range(len(H_CH))
        ]
        for hc2, (g0, gp) in enumerate(H_CH):
            for hc, (h0, hp) in enumerate(H_CH):
                nc.tensor.matmul(
                    ps2[hc2][0:gp, 0:n],
                    w2s[hc][0:hp, g0 : g0 + gp],
                    h1[hc][0:hp, 0:n],
                    start=(hc == 0),
                    stop=(hc == len(H_CH) - 1),
                )
        # h2 evictions on DVE (bias-add + relu) to keep ScalarE short
        h2 = []
        for hc2, (g0, gp) in enumerate(H_CH):
            h = hp_.tile([gp, BT], BF16, name=f"h2_{hc2}", tag=f"h2_{hc2}")
            nc.vector.tensor_scalar(
                h[:, 0:n], ps2[hc2][0:gp, 0:n], b2s[hc2][:, :], 0.0,
                mybir.AluOpType.add, mybir.AluOpType.max,
            )
            h2.append(h)

        # fc3: h2 @ W3 + b3 -> [10, n] (10-col stationary, LDW ~free);
        # store hidden-major — the host un-transposes.
        ps = ps3p.tile([NCLS, 512], F32, name="ps3", tag="f3")
        for hc, (h0, hp) in enumerate(H_CH):
            nc.tensor.matmul(
                ps[0:NCLS, 0:n],
                w3s[hc][0:hp, 0:NCLS],
                h2[hc][0:hp, 0:n],
                start=(hc == 0),
                stop=(hc == len(H_CH) - 1),
            )
        ob = obp.tile([NCLS, BT], F32, name="ob", tag="ob")
        nc.scalar.activation(
            ob[:, 0:n], ps[0:NCLS, 0:n], AFT.Identity, bias=b3s[:, :]
        )
        nc.sync.dma_start(out[:, c0 + off : c0 + off + n], ob[:, 0:n])

    for t in range(NBT):
        c0 = t * BT
        xs = xts.pop(t)
        mid = None
        if t + 2 < NBT:
            mid = lambda t=t: xts.__setitem__(t + 2, load_x(t + 2))
        if t == NBT - 1:
            # split the last tile to shorten the serial tail chain
            compute(xs, c0, 0, 256, mid=mid)
            compute(xs, c0, 256, 256)
        else:
            compute(xs, c0, 0, BT, mid=mid)


def _fold_w1(conv_w: np.ndarray, W1: np.ndarray) -> np.ndarray:
    """W1' = C @ W1 via the 9-tap sparse form: 9 scaled slice-adds."""
    W1m = W1.reshape(OHW, OHW, HID)
    out = np.zeros((IMG, IMG, HID), np.float32)
    for dy in range(3):
        for dx in range(3):
            out[dy : dy + OHW, dx : dx + OHW, :] += conv_w[dy, dx] * W1m
    return out.reshape(NPIX, HID)


_NC_CACHE: list = []


def _get_nc():
    if _NC_CACHE:
        return _NC_CACHE[0]
    nc = bass.Bass("TRN2", target_bir_lowering=False, debug=False)
    xt = nc.dram_tensor("xt", [NPIX, B_CORE], BF16, kind="ExternalInput").ap()
    wpk = nc.dram_tensor("wpk", [128, 3030], BF16, kind="ExternalInput").ap()
    bpk = nc.dram_tensor("bpk", [128, 7], F32, kind="ExternalInput").ap()
    out = nc.dram_tensor("out", [NCLS, B_CORE], F32, kind="ExternalOutput").ap()
    with tile.TileContext(nc) as tc:
        with ExitStack() as ctx:
            _emit(ctx, tc, xt, wpk, bpk, out)
    _legalize_single_wait(nc)
    _NC_CACHE.append(nc)
    return nc


def _in_maps(inputs: dict) -> list:
    x = np.asarray(inputs["x"], dtype=np.float32)
    assert x.shape == (B_FULL, NPIX), x.shape
    # pixel-major per-core layout: [8, 784, 4096] bf16 (zero-FLOP reshape)
    xtp = np.ascontiguousarray(
        x.reshape(N_CORES, B_CORE, NPIX).transpose(0, 2, 1)
    ).astype(ml_dtypes.bfloat16)
    bf = ml_dtypes.bfloat16
    w1f = _fold_w1(
        np.asarray(inputs["conv_w"], np.float32),
        np.asarray(inputs["W1"], np.float32),
    ).astype(bf)
    W2 = np.asarray(inputs["W2"], np.float32)
    W3 = np.asarray(inputs["W3"], np.float32)
    # packed weight tile: w1p chunks 0-5 | w1p6 (replicated) | w2 | w3
    wpk = np.zeros((128, 3030), bf)
    for pc in range(NFULL):
        wpk[:, pc * HID : (pc + 1) * HID] = w1f[pc * 128 : (pc + 1) * 128]
    for r in range(3):
        wpk[32 * r : 32 * r + 16, NFULL * HID : NFULL * HID + HID] = w1f[768:784]
    for hc, (h0, hp) in enumerate(H_CH):
        wpk[0:hp, 2100 + hc * HID : 2100 + (hc + 1) * HID] = W2[h0 : h0 + hp].astype(bf)
        wpk[0:hp, 3000 + hc * NCLS : 3000 + (hc + 1) * NCLS] = W3[h0 : h0 + hp].astype(bf)
    bpk = np.zeros((128, 7), np.float32)
    b1 = np.asarray(inputs["b1"], np.float32)
    b2 = np.asarray(inputs["b2"], np.float32)
    for hc, (h0, hp) in enumerate(H_CH):
        bpk[0:hp, hc] = b1[h0 : h0 + hp]
        bpk[0:hp, 3 + hc] = b2[h0 : h0 + hp]
    bpk[0:NCLS, 6] = np.asarray(inputs["b3"], np.float32)
    common = {"wpk": wpk, "bpk": bpk}
    return [{"xt": xtp[c], **common} for c in range(N_CORES)]


def kernel(**inputs) -> np.ndarray:
    nc = _get_nc()
    res = run_bass_kernel_spmd(nc, _in_maps(inputs), list(range(N_CORES)))
    return np.concatenate(
        [res.results[c]["out"].T for c in range(N_CORES)], axis=0
    )


if __name__ == "__main__":
    rng = np.random.default_rng(0)
    ins = {
        "x": rng.standard_normal((B_FULL, NPIX), dtype=np.float32),
        "conv_w": rng.standard_normal((3, 3), dtype=np.float32) * 0.1,
        "W1": rng.standard_normal((FLAT, HID), dtype=np.float32) * 0.04,
        "b1": np.zeros(HID, np.float32),
        "W2": rng.standard_normal((HID, HID), dtype=np.float32) * 0.06,
        "b2": np.zeros(HID, np.float32),
        "W3": rng.standard_normal((HID, NCLS), dtype=np.float32) * 0.06,
        "b3": np.zeros(NCLS, np.float32),
    }
    y = kernel(**ins)
    # numpy reference with explicit conv
    from numpy.lib.stride_tricks import sliding_window_view

    img = ins["x"].reshape(-1, IMG, IMG)
    win = sliding_window_view(img, (3, 3), axis=(1, 2))
    conv = np.einsum("bijkl,kl->bij", win, ins["conv_w"]).reshape(-1, FLAT)
    h = np.maximum(conv @ ins["W1"] + ins["b1"], 0)
    h = np.maximum(h @ ins["W2"] + ins["b2"], 0)
    ref = h @ ins["W3"] + ins["b3"]
    err = np.abs(y - ref).max() / (np.abs(ref).max() + 1e-9)
    print("max rel err vs numpy:", err)
